# revision 38
# baseline (speedup 1.0000x reference)
"""Trainium2 Bass kernel for BasicBlockIMCFlow (quantized ResNet basic block).

Math (exact integer arithmetic; fp32/int16/fp8 carriers):
  t    = sat_i16(rne(x*256))   computed on the HOST (np.rint is rne; exact)
                               and shipped as int16 -> halves the input DMA
                               and removes one full elementwise pass
  q1   = clip((t+512)>>10, 0, 15)  also computed on the HOST (exact integer
                               math) and shipped as the zero-padded 66x66 fp8
                               conv-input plane -> deletes all of stage A
                               (quantize ops + border memsets) on-device
  h1   = conv3x3(q1, w1)
  q2   = clip(rne_i16(h1*s1/2048 + (2*b1+1)/4096), 0, 15)
  h2   = conv3x3(q2, w2)
  oi   = sat_i16(h2*s2 + b2 + t)                  (== to_int16(h2*s2+b2+x_int))
  out  = oi / 256  (host side)

Engine float->int16 conversions round-to-nearest-even and saturate (verified
on hardware) == jnp.round + int16 clip; the 2^-11 / (2b+1)/4096 bias guards
turn round-to-nearest into the required floor with no representable ties.
(relu commutes with rne: rne(max(v,0)) == max(rne(v),0), so the 0-clip can
ride in the q1 pack op on either engine.)

Convs are fp8 DoubleRow matmuls (2 k-tiles per pass, 2x contraction): the two
images of a pair are stacked on the 128 partitions with block-diagonal
weights; tap pairs are fed from one zero-padded [66,66] fp8 buffer via 4D
overlapping access patterns. Per 8-output-row chunk (512 psum cols = 1 bank):
4 DoubleRow passes (8 taps) + 1 regular pass (tap (2,2)) = 5 matmuls; the PE
streams ~222 ns per 512-col pass (1 col/cycle, LDWEIGHTS hidden), wait-free
mid-stream — the matmul stream is within ~5% of the hw column-rate floor.

Head: per-queue DMA runs ~90 GB/s (~270 aggregate over the 3 trigger-capable
queues); q1(pair0) lands first (first piece split 3 ways), so conv1 starts
~5us after the first trigger with no on-device dependency but the DMA itself.
Steady state ships ~0.9 MB per pair of q1 planes + t16 spread over all three
queues, interleaved with the output stores.
Tail: conv2 runs per 512-col chunk (1 psum bank, 4-buf rotation); stores of
pairs 0-2 ride the otherwise-idle gpsimd queue, pair 3's rotate across
queues with the final chunk split across sync+gpsimd.

Schedule: software-pipelined sweeps. Sweep p runs conv2(pair p) interleaved
with conv1(pair p+1) on the PE while stage A of pair p+2 runs on Act/DVE.
qp border zeroing happens only on first use of each rotating pool buffer.

Data parallel: batch 64 -> 8 images/core; output is DMA'd as int16 and
divided by 256 on the host (exact).
"""

import os

import numpy as np

_CACHE = {}

B, C, H, W = 64, 64, 64, 64
HW = H * W                    # 4096
PW = W + 2                    # 66
N_CORES = 8
IMG_PER_CORE = B // N_CORES   # 8
PAIRS = IMG_PER_CORE // 2     # 4
CHUNK_ROWS = 8                # output rows per matmul chunk (512 cols = bank)
NCHUNK = H // CHUNK_ROWS      # 8
CHUNK_N = CHUNK_ROWS * W      # 512
GEN_CHUNKS = 2                # chunks per conv1 psum generation tile
NGEN = NCHUNK // GEN_CHUNKS   # 4
GEN_N = GEN_CHUNKS * CHUNK_N  # 1024
NQ = 4                        # stage A quarters (steady-state pairs)
Q_ROWS = H // NQ              # 16

# pair-0 head row-groups: group i covers x rows [lo, hi] (inclusive); group i
# covers through row 8i+8 so conv chunk j (reads x rows 8j-1..8j+8) depends
# only on groups <= j.
GROUPS0 = [(0, 8)] + [(8 * i + 1, 8 * i + 8) for i in range(1, 7)] + [(57, 63)]
# pair-1 uses the same 8-row groups (conv1 gen g needs groups <= 2g+1).
GROUPS1 = GROUPS0

TAP_PAIRS = [((0, 0), (0, 1)), ((1, 1), (1, 2)), ((2, 0), (2, 1)),
             ((0, 2), (1, 0))]
TAP_SINGLE = (2, 2)


def _build_nc():
    import concourse.bacc as bacc
    import concourse.tile as tile
    import concourse.mybir as mybir
    from contextlib import ExitStack

    f32 = mybir.dt.float32
    i16 = mybir.dt.int16
    fp8 = mybir.dt.float8e4
    Alu = mybir.AluOpType
    Act = mybir.ActivationFunctionType
    DR = mybir.MatmulPerfMode.DoubleRow

    nc = bacc.Bacc()

    t_d = nc.dram_tensor("t", [IMG_PER_CORE, C, HW], i16, kind="ExternalInput")
    q1_d = nc.dram_tensor("q1p", [IMG_PER_CORE, C, (H + 2) * PW], fp8,
                          kind="ExternalInput")
    w1_d = nc.dram_tensor("w1t", [128, 9 * 128], fp8, kind="ExternalInput")
    w2_d = nc.dram_tensor("w2t", [128, 9 * 128], fp8, kind="ExternalInput")
    pp_d = nc.dram_tensor("pp", [128, 6], f32, kind="ExternalInput")
    out_d = nc.dram_tensor("out", [IMG_PER_CORE, C, HW], i16,
                           kind="ExternalOutput")

    with tile.TileContext(nc) as tc:
        with ExitStack() as ctx:
            singles = ctx.enter_context(tc.tile_pool(name="singles", bufs=1))
            tp = ctx.enter_context(tc.tile_pool(name="tp", bufs=3))
            qp1p = ctx.enter_context(tc.tile_pool(name="qp1p", bufs=2))
            qp2p = ctx.enter_context(tc.tile_pool(name="qp2p", bufs=2))
            g2p = ctx.enter_context(tc.tile_pool(name="g2p", bufs=5))
            up = ctx.enter_context(tc.tile_pool(name="up", bufs=5))
            otp = ctx.enter_context(tc.tile_pool(name="otp", bufs=6))
            ps1 = ctx.enter_context(tc.tile_pool(name="ps1", bufs=2,
                                                 space="PSUM"))
            ps2 = ctx.enter_context(tc.tile_pool(name="ps2", bufs=4,
                                                 space="PSUM"))

            w1b = singles.tile([128, 9, 128], fp8, tag="w1b")
            w2b = singles.tile([128, 9, 128], fp8, tag="w2b")
            pp = singles.tile([128, 6], f32, tag="pp")
            sB, bB = pp[:, 0:1], pp[:, 1:2]
            sC, bC = pp[:, 2:3], pp[:, 3:4]
            eps11 = pp[:, 4:5]   # 2^-11
            zero_c = pp[:, 5:6]  # 0.0

            def conv_rhs(qp, r, kyA, kxA, delta):
                full = qp[:, :, :]
                ap = full.copy()
                VP = type(ap.ap)
                ap.ap = VP([[full.ap[0][0], 128], [delta, 2],
                            [PW, CHUNK_ROWS], [1, W]])
                ap.offset = full.offset + (r + kyA) * PW + kxA
                return ap

            def conv_chunk(ps_tile, col0, wb, qp, j):
                """5 passes (4 DoubleRow + 1 single-tap) for rows 8j..8j+7."""
                r = j * CHUNK_ROWS
                dst = ps_tile[:, col0:col0 + CHUNK_N]
                for g, ((kyA, kxA), (kyB, kxB)) in enumerate(TAP_PAIRS):
                    delta = (kyB - kyA) * PW + (kxB - kxA)
                    nc.tensor.matmul(
                        dst, wb[:, 2 * g:2 * g + 2, :],
                        conv_rhs(qp, r, kyA, kxA, delta),
                        start=(g == 0), stop=False, perf_mode=DR)
                ky, kx = TAP_SINGLE
                nc.tensor.matmul(
                    dst, wb[:, 8, :],
                    qp[:, r + ky:r + ky + CHUNK_ROWS, kx:kx + W],
                    start=False, stop=True)

            # ---- per-pair state and stage helpers ----
            def x_pair_ap(p):
                return t_d[2 * p:2 * p + 2, :, :].rearrange(
                    "b c n -> (b c) n")

            def q1_pair_ap(p):
                return q1_d[2 * p:2 * p + 2, :, :].rearrange(
                    "b c n -> (b c) n")

            def dma_q1(st, engines=None, pieces=2):
                qsrc = q1_pair_ap(st["p"])
                qdst = st["qp1"].rearrange("p a b -> p (a b)")
                if engines is None:
                    engines = [nc.sync, nc.gpsimd]
                n = (H + 2) * PW
                step = -(-n // pieces)
                for i in range(pieces):
                    cs = slice(i * step, min((i + 1) * step, n))
                    engines[i].dma_start(out=qdst[:, cs], in_=qsrc[:, cs])

            def dma_x(st, engines=None, quarters=range(NQ)):
                if engines is None:
                    engines = [nc.sync, nc.scalar, nc.gpsimd, nc.sync]
                for q in quarters:
                    cs = slice(q * Q_ROWS * W, (q + 1) * Q_ROWS * W)
                    engines[q].dma_start(out=st["t16"][:, cs],
                                         in_=x_pair_ap(st["p"])[:, cs])

            def borders(qp):
                nc.vector.memset(qp[:, 0, :], 0.0)
                nc.vector.memset(qp[:, H + 1, :], 0.0)
                nc.vector.memset(qp[:, 1:H + 1, 0], 0.0)
                nc.vector.memset(qp[:, 1:H + 1, PW - 1], 0.0)

            def conv1_gen(st, g):
                pst = ps1.tile([128, GEN_N], f32, tag="ps1")
                conv_chunk(pst, 0, w1b, st["qp1"], g * GEN_CHUNKS)
                conv_chunk(pst, CHUNK_N, w1b, st["qp1"], g * GEN_CHUNKS + 1)
                # g2 = rne_i16(h1*s1/2048 + (2 b1 + 1)/4096)
                g2 = g2p.tile([128, GEN_N], i16, tag="g2")
                nc.vector.tensor_scalar(out=g2, in0=pst, scalar1=sB,
                                        scalar2=bB, op0=Alu.mult, op1=Alu.add)
                # q2 = clip(g2, 0, 15) -> fp8 strided interior rows
                r0 = g * GEN_CHUNKS * CHUNK_ROWS
                dq = st["qp2"][:, 1 + r0:1 + r0 + GEN_CHUNKS * CHUNK_ROWS,
                               1:W + 1]
                nc.vector.tensor_scalar(out=dq, in0=g2, scalar1=0, scalar2=15,
                                        op0=Alu.max, op1=Alu.min)

            def conv2_chunk(st, j, store_engines=None):
                """conv2 on one 512-col chunk (1 psum bank) + post; the
                int16 results of chunk pairs (2j, 2j+1) share one 1024-col
                ot tile and are stored together (2KB-per-partition DMA runs
                instead of 1KB)."""
                pst = ps2.tile([128, CHUNK_N], f32, tag="ps2")
                conv_chunk(pst, 0, w2b, st["qp2"], j)
                i0 = st["p"] * 2
                u = up.tile([128, CHUNK_N], f32, tag="u")
                if j % 2 == 0:
                    ot2 = otp.tile([128, 2 * CHUNK_N], i16, tag="ot",
                                   name=f"ot_{st['p']}_{j}")
                    st["ot"] = ot2
                ot = st["ot"]
                oth = ot[:, (j % 2) * CHUNK_N:(j % 2 + 1) * CHUNK_N]
                gs0 = j * CHUNK_N
                # u = h2*s2 + b2
                nc.scalar.activation(out=u, in_=pst, func=Act.Identity,
                                     bias=bC, scale=sC)
                # oi = sat_i16(t + u)
                nc.vector.scalar_tensor_tensor(
                    out=oth, in0=st["t16"][:, gs0:gs0 + CHUNK_N],
                    scalar=0.0, in1=u, op0=Alu.add, op1=Alu.add)
                if j % 2 == 1:
                    if store_engines is None:
                        store_engines = [nc.gpsimd if (j // 2) % 2 == 0
                                         else nc.scalar]
                    b0 = (j - 1) * CHUNK_N
                    n = 2 * CHUNK_N // len(store_engines)
                    for h, eng in enumerate(store_engines):
                        gs = slice(b0 + h * n, b0 + (h + 1) * n)
                        od = out_d[i0:i0 + 2, :, gs].rearrange(
                            "b c n -> (b c) n")
                        eng.dma_start(out=od, in_=ot[:, h * n:(h + 1) * n])

            def new_state(p):
                t16 = tp.tile([128, HW], i16, tag="t16")
                qp1 = qp1p.tile([128, H + 2, PW], fp8, tag="qp1")
                qp2 = qp2p.tile([128, H + 2, PW], fp8, tag="qp2")
                return {"p": p, "t16": t16, "qp1": qp1, "qp2": qp2}

            # ---- prologue ----
            states = [new_state(p) for p in range(PAIRS)]
            s0, s1 = states[0], states[1]
            xs0 = s0["t16"]
            xs1 = s1["t16"]
            x0 = x_pair_ap(0)
            x1 = x_pair_ap(1)
            w1r = w1_d.rearrange("p (t m) -> p t m", m=128)
            w2r = w2_d.rearrange("p (t m) -> p t m", m=128)

            def xdma(eng, xs, xp, r_lo, r_hi, c_lo=0, c_hi=None):
                lo = r_lo * W + c_lo
                hi = (r_hi + 1) * W if c_hi is None else r_lo * W + c_hi
                eng.dma_start(out=xs[:, lo:hi], in_=xp[:, lo:hi])

            # q1(pair0) lands first: piece 0 (padded rows 0-9) split 3 ways,
            # then the rest; weights and t16/q1 of pair 1 ride behind
            # (note: PE warm-up matmuls to burn the p-state ramp are a trap:
            # 10 passes measured 91.0us once, then 109.5us throttled — the
            # sustained-load throttle boundary sits right there; not worth it)
            q0src = q1_pair_ap(0)
            q0dst = s0["qp1"].rearrange("p a b -> p (a b)")
            q1src = q1_pair_ap(1)
            q1dst = s1["qp1"].rearrange("p a b -> p (a b)")

            def qdma(eng, dst, src, lo, hi):
                eng.dma_start(out=dst[:, lo:hi], in_=src[:, lo:hi])

            qdma(nc.sync, q0dst, q0src, 0, 220)
            qdma(nc.scalar, q0dst, q0src, 220, 440)
            qdma(nc.gpsimd, q0dst, q0src, 440, 660)      # padded rows 0-9
            nc.gpsimd.dma_start(out=w1b, in_=w1r)        # w1
            nc.sync.dma_start(out=pp, in_=pp_d[:])
            qdma(nc.sync, q0dst, q0src, 660, 1914)       # rows 10-28
            qdma(nc.scalar, q0dst, q0src, 1914, 3168)    # rows 29-47
            qdma(nc.gpsimd, q0dst, q0src, 3168, 4356)    # rows 48-65
            xdma(nc.sync, xs0, x0, 0, 20)                # t16(s0) rows 0-20
            xdma(nc.scalar, xs0, x0, 21, 41)
            nc.gpsimd.dma_start(out=w2b, in_=w2r)        # w2
            qdma(nc.sync, q1dst, q1src, 0, 1452)         # q1(s1) rows 0-21
            qdma(nc.scalar, q1dst, q1src, 1452, 2904)
            qdma(nc.gpsimd, q1dst, q1src, 2904, 4356)
            xdma(nc.gpsimd, xs0, x0, 42, 63)
            xdma(nc.sync, xs1, x1, 0, 20)                # t16(s1)
            xdma(nc.scalar, xs1, x1, 21, 41)
            xdma(nc.gpsimd, xs1, x1, 42, 63)

            borders(s0["qp2"])
            borders(s1["qp2"])
            for g in range(NGEN):
                conv1_gen(s0, g)

            # ---- sweeps ----
            for p in range(PAIRS):
                cur = states[p]
                nx1 = states[p + 1] if p + 1 < PAIRS else None
                nx2 = states[p + 2] if p + 2 < PAIRS else None
                if nx2 is not None:
                    # qp2 pool buffers rotate with bufs=2: pair p+2 reuses
                    # pair p's buffer whose pads are still zero -> no borders.
                    # qp1 including pads arrives fully by DMA.
                    dma_x(nx2)
                    dma_q1(nx2)
                last = (p == PAIRS - 1)
                rot = [[nc.gpsimd], [nc.sync], [nc.scalar]]
                for j in range(NCHUNK):
                    if last and j % 2 == 1:
                        se = ([nc.sync, nc.gpsimd] if j == NCHUNK - 1
                              else rot[(j // 2) % 3])
                    else:
                        se = None
                    conv2_chunk(cur, j, store_engines=se)
                    if j % 2 == 1:
                        g = j // 2
                        if nx1 is not None:
                            conv1_gen(nx1, g)

    nc.compile()
    return nc


def _get_nc():
    if "nc" not in _CACHE:
        _CACHE["nc"] = _build_nc()
    return _CACHE["nc"]


def _prep_host_inputs(inputs):
    import concourse.mybir as mybir

    fp8np = mybir.dt.np(mybir.dt.float8e4)

    x = np.asarray(inputs["x"], dtype=np.float32).reshape(B, C, HW)
    t = np.clip(np.rint(x * 256.0), -32768.0, 32767.0).astype(np.int16)
    t = np.ascontiguousarray(t)
    # q1 = clip(floor((t+512)/1024), 0, 15), host-padded to the 66x66 conv
    # input plane (exact integer math; fp8e4 holds 0..15 exactly)
    q1 = np.clip((t.astype(np.int32) + 512) >> 10, 0, 15)
    q1p = np.zeros((B, C, H + 2, PW), np.float32)
    q1p[:, :, 1:H + 1, 1:W + 1] = q1.reshape(B, C, H, W)
    q1p = np.ascontiguousarray(
        q1p.reshape(B, C, (H + 2) * PW).astype(fp8np))

    def wprep(w):
        wt = np.ascontiguousarray(w, dtype=np.float32).reshape(C, C, 3, 3)
        wt = wt.transpose(1, 0, 2, 3)                  # [in, out, ky, kx]
        taps = [kk for pair in TAP_PAIRS for kk in pair] + [TAP_SINGLE]
        out = np.zeros((128, 9, 128), np.float32)
        for t, (ky, kx) in enumerate(taps):
            out[0:64, t, 0:64] = wt[:, :, ky, kx]
            out[64:128, t, 64:128] = wt[:, :, ky, kx]
        return np.ascontiguousarray(out.reshape(128, 9 * 128).astype(fp8np))

    w1t = wprep(inputs["w1"])
    w2t = wprep(inputs["w2"])

    s1 = np.asarray(inputs["bn1_scale"], dtype=np.float64)
    b1 = np.asarray(inputs["bn1_bias"], dtype=np.float64)
    s2 = np.asarray(inputs["bn2_scale"], dtype=np.float64)
    b2 = np.asarray(inputs["bn2_bias"], dtype=np.float64)
    # all exact dyadic rationals -> float32 conversion is exact
    sB = (s1 * 2.0 ** -11).astype(np.float32)
    bB = ((2.0 * b1 + 1.0) * 2.0 ** -12).astype(np.float32)
    sC = s2.astype(np.float32)
    bC = b2.astype(np.float32)
    eps = np.full(64, 2.0 ** -11, dtype=np.float32)
    zer = np.zeros(64, dtype=np.float32)
    pp = np.stack([sB, bB, sC, bC, eps, zer], axis=1)      # [64, 6]
    pp = np.ascontiguousarray(np.concatenate([pp, pp], axis=0))  # [128, 6]

    return t, q1p, w1t, w2t, pp


def kernel(**inputs):
    from concourse.bass_utils import run_bass_kernel_spmd

    t, q1p, w1t, w2t, pp = _prep_host_inputs(inputs)
    nc = _get_nc()
    in_maps = []
    for i in range(N_CORES):
        sl = slice(i * IMG_PER_CORE, (i + 1) * IMG_PER_CORE)
        in_maps.append({"t": np.ascontiguousarray(t[sl]),
                        "q1p": np.ascontiguousarray(q1p[sl]),
                        "w1t": w1t, "w2t": w2t, "pp": pp})

    trace = bool(int(os.environ.get("KERNEL_TRACE", "0")))
    res = run_bass_kernel_spmd(nc, in_maps, core_ids=list(range(N_CORES)),
                               trace=trace)
    _CACHE["last_results"] = res
    out = np.concatenate([r["out"] for r in res.results], axis=0)
    return (out.reshape(B, C, H, W).astype(np.float32)) / 256.0


# revision 39
# speedup vs baseline: 1.0209x; 1.0209x over previous
"""Trainium2 Bass kernel for BasicBlockIMCFlow (quantized ResNet basic block).

Math (exact integer arithmetic; fp32/int16/fp8 carriers):
  t    = sat_i16(rne(x*256))   computed on the HOST (np.rint is rne; exact)
                               and shipped as int16 -> halves the input DMA
                               and removes one full elementwise pass
  q1   = clip((t+512)>>10, 0, 15)  also computed on the HOST (exact integer
                               math) and shipped as the zero-padded 66x66 fp8
                               conv-input plane -> deletes all of stage A
                               (quantize ops + border memsets) on-device
  h1   = conv3x3(q1, w1)
  q2   = clip(rne_i16(h1*s1/2048 + (2*b1+1)/4096), 0, 15)
  h2   = conv3x3(q2, w2)
  oi   = sat_i16(h2*s2 + b2 + t)                  (== to_int16(h2*s2+b2+x_int))
  out  = oi / 256  (host side)

Engine float->int16 conversions round-to-nearest-even and saturate (verified
on hardware) == jnp.round + int16 clip; the 2^-11 / (2b+1)/4096 bias guards
turn round-to-nearest into the required floor with no representable ties.
(relu commutes with rne: rne(max(v,0)) == max(rne(v),0), so the 0-clip can
ride in the q1 pack op on either engine.)

Convs are fp8 DoubleRow matmuls (2 k-tiles per pass, 2x contraction): the two
images of a pair are stacked on the 128 partitions with block-diagonal
weights; tap pairs are fed from one zero-padded [66,66] fp8 buffer via 4D
overlapping access patterns. Per 8-output-row chunk (512 psum cols = 1 bank):
4 DoubleRow passes (8 taps) + 1 regular pass (tap (2,2)) = 5 matmuls; the PE
streams ~222 ns per 512-col pass (1 col/cycle, LDWEIGHTS hidden), wait-free
mid-stream — the matmul stream is within ~5% of the hw column-rate floor.

Head: per-queue DMA runs ~90 GB/s (~270 aggregate over the 3 trigger-capable
queues); q1(pair0) lands first (first piece split 3 ways), so conv1 starts
~5us after the first trigger with no on-device dependency but the DMA itself.
Steady state ships ~0.9 MB per pair of q1 planes + t16 spread over all three
queues, interleaved with the output stores.
Tail: conv2 runs per 512-col chunk (1 psum bank, 4-buf rotation); stores of
pairs 0-2 ride the otherwise-idle gpsimd queue, pair 3's rotate across
queues with the final chunk split across sync+gpsimd.

Schedule: software-pipelined sweeps. Sweep p runs conv2(pair p) interleaved
with conv1(pair p+1) on the PE while stage A of pair p+2 runs on Act/DVE.
qp border zeroing happens only on first use of each rotating pool buffer.

Data parallel: batch 64 -> 8 images/core; output is DMA'd as int16 and
divided by 256 on the host (exact).
"""

import os

import numpy as np

_CACHE = {}

B, C, H, W = 64, 64, 64, 64
HW = H * W                    # 4096
PW = W + 2                    # 66
N_CORES = 8
IMG_PER_CORE = B // N_CORES   # 8
PAIRS = IMG_PER_CORE // 2     # 4
CHUNK_ROWS = 8                # output rows per matmul chunk (512 cols = bank)
NCHUNK = H // CHUNK_ROWS      # 8
CHUNK_N = CHUNK_ROWS * W      # 512
GEN_CHUNKS = 2                # chunks per conv1 psum generation tile
NGEN = NCHUNK // GEN_CHUNKS   # 4
GEN_N = GEN_CHUNKS * CHUNK_N  # 1024
NQ = 4                        # stage A quarters (steady-state pairs)
Q_ROWS = H // NQ              # 16

# pair-0 head row-groups: group i covers x rows [lo, hi] (inclusive); group i
# covers through row 8i+8 so conv chunk j (reads x rows 8j-1..8j+8) depends
# only on groups <= j.
GROUPS0 = [(0, 8)] + [(8 * i + 1, 8 * i + 8) for i in range(1, 7)] + [(57, 63)]
# pair-1 uses the same 8-row groups (conv1 gen g needs groups <= 2g+1).
GROUPS1 = GROUPS0

TAP_PAIRS = [((0, 0), (0, 1)), ((1, 1), (1, 2)), ((2, 0), (2, 1)),
             ((0, 2), (1, 0))]
TAP_SINGLE = (2, 2)


def _build_nc():
    import concourse.bacc as bacc
    import concourse.tile as tile
    import concourse.mybir as mybir
    from contextlib import ExitStack

    f32 = mybir.dt.float32
    i16 = mybir.dt.int16
    fp8 = mybir.dt.float8e4
    Alu = mybir.AluOpType
    Act = mybir.ActivationFunctionType
    DR = mybir.MatmulPerfMode.DoubleRow

    nc = bacc.Bacc()

    t_d = nc.dram_tensor("t", [IMG_PER_CORE, C, HW], i16, kind="ExternalInput")
    q1_d = nc.dram_tensor("q1p", [IMG_PER_CORE, C, (H + 2) * PW], fp8,
                          kind="ExternalInput")
    w1_d = nc.dram_tensor("w1t", [128, 9 * 128], fp8, kind="ExternalInput")
    w2_d = nc.dram_tensor("w2t", [128, 9 * 128], fp8, kind="ExternalInput")
    pp_d = nc.dram_tensor("pp", [128, 6], f32, kind="ExternalInput")
    out_d = nc.dram_tensor("out", [IMG_PER_CORE, C, HW], i16,
                           kind="ExternalOutput")

    with tile.TileContext(nc) as tc:
        with ExitStack() as ctx:
            singles = ctx.enter_context(tc.tile_pool(name="singles", bufs=1))
            tp = ctx.enter_context(tc.tile_pool(name="tp", bufs=3))
            qp1p = ctx.enter_context(tc.tile_pool(name="qp1p", bufs=2))
            qp2p = ctx.enter_context(tc.tile_pool(name="qp2p", bufs=2))
            g2p = ctx.enter_context(tc.tile_pool(name="g2p", bufs=5))
            up = ctx.enter_context(tc.tile_pool(name="up", bufs=5))
            otp = ctx.enter_context(tc.tile_pool(name="otp", bufs=6))
            ps1 = ctx.enter_context(tc.tile_pool(name="ps1", bufs=2,
                                                 space="PSUM"))
            ps2 = ctx.enter_context(tc.tile_pool(name="ps2", bufs=4,
                                                 space="PSUM"))

            w1b = singles.tile([128, 9, 128], fp8, tag="w1b")
            w2b = singles.tile([128, 9, 128], fp8, tag="w2b")
            pp = singles.tile([128, 6], f32, tag="pp")
            sB, bB = pp[:, 0:1], pp[:, 1:2]
            sC, bC = pp[:, 2:3], pp[:, 3:4]
            eps11 = pp[:, 4:5]   # 2^-11
            zero_c = pp[:, 5:6]  # 0.0

            def conv_rhs(qp, r, kyA, kxA, delta):
                full = qp[:, :, :]
                ap = full.copy()
                VP = type(ap.ap)
                ap.ap = VP([[full.ap[0][0], 128], [delta, 2],
                            [PW, CHUNK_ROWS], [1, W]])
                ap.offset = full.offset + (r + kyA) * PW + kxA
                return ap

            def conv_chunk(ps_tile, col0, wb, qp, j):
                """5 passes (4 DoubleRow + 1 single-tap) for rows 8j..8j+7."""
                r = j * CHUNK_ROWS
                dst = ps_tile[:, col0:col0 + CHUNK_N]
                for g, ((kyA, kxA), (kyB, kxB)) in enumerate(TAP_PAIRS):
                    delta = (kyB - kyA) * PW + (kxB - kxA)
                    nc.tensor.matmul(
                        dst, wb[:, 2 * g:2 * g + 2, :],
                        conv_rhs(qp, r, kyA, kxA, delta),
                        start=(g == 0), stop=False, perf_mode=DR)
                ky, kx = TAP_SINGLE
                nc.tensor.matmul(
                    dst, wb[:, 8, :],
                    qp[:, r + ky:r + ky + CHUNK_ROWS, kx:kx + W],
                    start=False, stop=True)

            # ---- per-pair state and stage helpers ----
            def x_pair_ap(p):
                return t_d[2 * p:2 * p + 2, :, :].rearrange(
                    "b c n -> (b c) n")

            def q1_pair_ap(p):
                return q1_d[2 * p:2 * p + 2, :, :].rearrange(
                    "b c n -> (b c) n")

            def dma_q1(st, engines=None, pieces=2):
                qsrc = q1_pair_ap(st["p"])
                qdst = st["qp1"].rearrange("p a b -> p (a b)")
                if engines is None:
                    engines = [nc.sync, nc.gpsimd]
                n = (H + 2) * PW
                step = -(-n // pieces)
                for i in range(pieces):
                    cs = slice(i * step, min((i + 1) * step, n))
                    engines[i].dma_start(out=qdst[:, cs], in_=qsrc[:, cs])

            def dma_x(st, engines=None, quarters=range(NQ)):
                if engines is None:
                    engines = [nc.sync, nc.scalar, nc.gpsimd, nc.sync]
                for q in quarters:
                    cs = slice(q * Q_ROWS * W, (q + 1) * Q_ROWS * W)
                    engines[q].dma_start(out=st["t16"][:, cs],
                                         in_=x_pair_ap(st["p"])[:, cs])

            def borders(qp):
                nc.vector.memset(qp[:, 0, :], 0.0)
                nc.vector.memset(qp[:, H + 1, :], 0.0)
                nc.vector.memset(qp[:, 1:H + 1, 0], 0.0)
                nc.vector.memset(qp[:, 1:H + 1, PW - 1], 0.0)

            def conv1_gen(st, g):
                pst = ps1.tile([128, GEN_N], f32, tag="ps1")
                conv_chunk(pst, 0, w1b, st["qp1"], g * GEN_CHUNKS)
                conv_chunk(pst, CHUNK_N, w1b, st["qp1"], g * GEN_CHUNKS + 1)
                # g2 = rne_i16(h1*s1/2048 + (2 b1 + 1)/4096)
                g2 = g2p.tile([128, GEN_N], i16, tag="g2")
                nc.vector.tensor_scalar(out=g2, in0=pst, scalar1=sB,
                                        scalar2=bB, op0=Alu.mult, op1=Alu.add)
                # q2 = clip(g2, 0, 15) -> fp8 strided interior rows
                r0 = g * GEN_CHUNKS * CHUNK_ROWS
                dq = st["qp2"][:, 1 + r0:1 + r0 + GEN_CHUNKS * CHUNK_ROWS,
                               1:W + 1]
                nc.vector.tensor_scalar(out=dq, in0=g2, scalar1=0, scalar2=15,
                                        op0=Alu.max, op1=Alu.min)

            def conv2_chunk(st, j, store_engines=None):
                """conv2 on one 512-col chunk (1 psum bank) + post; the
                int16 results of chunk pairs (2j, 2j+1) share one 1024-col
                ot tile and are stored together (2KB-per-partition DMA runs
                instead of 1KB)."""
                pst = ps2.tile([128, CHUNK_N], f32, tag="ps2")
                conv_chunk(pst, 0, w2b, st["qp2"], j)
                i0 = st["p"] * 2
                u = up.tile([128, CHUNK_N], f32, tag="u")
                if j % 2 == 0:
                    ot2 = otp.tile([128, 2 * CHUNK_N], i16, tag="ot",
                                   name=f"ot_{st['p']}_{j}")
                    st["ot"] = ot2
                ot = st["ot"]
                oth = ot[:, (j % 2) * CHUNK_N:(j % 2 + 1) * CHUNK_N]
                gs0 = j * CHUNK_N
                # u = h2*s2 + b2
                nc.scalar.activation(out=u, in_=pst, func=Act.Identity,
                                     bias=bC, scale=sC)
                # oi = sat_i16(t + u)
                nc.vector.scalar_tensor_tensor(
                    out=oth, in0=st["t16"][:, gs0:gs0 + CHUNK_N],
                    scalar=0.0, in1=u, op0=Alu.add, op1=Alu.add)
                if j % 2 == 1:
                    if store_engines is None:
                        store_engines = [nc.gpsimd if (j // 2) % 2 == 0
                                         else nc.sync]
                    b0 = (j - 1) * CHUNK_N
                    n = 2 * CHUNK_N // len(store_engines)
                    for h, eng in enumerate(store_engines):
                        gs = slice(b0 + h * n, b0 + (h + 1) * n)
                        od = out_d[i0:i0 + 2, :, gs].rearrange(
                            "b c n -> (b c) n")
                        eng.dma_start(out=od, in_=ot[:, h * n:(h + 1) * n])

            def new_state(p):
                t16 = tp.tile([128, HW], i16, tag="t16")
                qp1 = qp1p.tile([128, H + 2, PW], fp8, tag="qp1")
                qp2 = qp2p.tile([128, H + 2, PW], fp8, tag="qp2")
                return {"p": p, "t16": t16, "qp1": qp1, "qp2": qp2}

            # ---- prologue ----
            states = [new_state(p) for p in range(PAIRS)]
            s0, s1 = states[0], states[1]
            xs0 = s0["t16"]
            xs1 = s1["t16"]
            x0 = x_pair_ap(0)
            x1 = x_pair_ap(1)
            w1r = w1_d.rearrange("p (t m) -> p t m", m=128)
            w2r = w2_d.rearrange("p (t m) -> p t m", m=128)

            def xdma(eng, xs, xp, r_lo, r_hi, c_lo=0, c_hi=None):
                lo = r_lo * W + c_lo
                hi = (r_hi + 1) * W if c_hi is None else r_lo * W + c_hi
                eng.dma_start(out=xs[:, lo:hi], in_=xp[:, lo:hi])

            # q1(pair0) lands first: piece 0 (padded rows 0-9) split 3 ways,
            # then the rest; weights and t16/q1 of pair 1 ride behind
            # (note: PE warm-up matmuls to burn the p-state ramp are a trap:
            # 10 passes measured 91.0us once, then 109.5us throttled — the
            # sustained-load throttle boundary sits right there; not worth it)
            q0src = q1_pair_ap(0)
            q0dst = s0["qp1"].rearrange("p a b -> p (a b)")
            q1src = q1_pair_ap(1)
            q1dst = s1["qp1"].rearrange("p a b -> p (a b)")

            def qdma(eng, dst, src, lo, hi):
                eng.dma_start(out=dst[:, lo:hi], in_=src[:, lo:hi])

            qdma(nc.sync, q0dst, q0src, 0, 220)
            qdma(nc.scalar, q0dst, q0src, 220, 440)
            qdma(nc.gpsimd, q0dst, q0src, 440, 660)      # padded rows 0-9
            nc.gpsimd.dma_start(out=w1b, in_=w1r)        # w1
            nc.sync.dma_start(out=pp, in_=pp_d[:])
            qdma(nc.sync, q0dst, q0src, 660, 1914)       # rows 10-28
            qdma(nc.scalar, q0dst, q0src, 1914, 3168)    # rows 29-47
            qdma(nc.gpsimd, q0dst, q0src, 3168, 4356)    # rows 48-65
            xdma(nc.sync, xs0, x0, 0, 20)                # t16(s0) rows 0-20
            xdma(nc.scalar, xs0, x0, 21, 41)
            nc.gpsimd.dma_start(out=w2b, in_=w2r)        # w2
            qdma(nc.sync, q1dst, q1src, 0, 1452)         # q1(s1) rows 0-21
            qdma(nc.scalar, q1dst, q1src, 1452, 2904)
            qdma(nc.gpsimd, q1dst, q1src, 2904, 4356)
            xdma(nc.gpsimd, xs0, x0, 42, 63)
            xdma(nc.sync, xs1, x1, 0, 20)                # t16(s1)
            xdma(nc.scalar, xs1, x1, 21, 41)
            xdma(nc.gpsimd, xs1, x1, 42, 63)

            borders(s0["qp2"])
            borders(s1["qp2"])
            for g in range(NGEN):
                conv1_gen(s0, g)

            # ---- sweeps ----
            for p in range(PAIRS):
                cur = states[p]
                nx1 = states[p + 1] if p + 1 < PAIRS else None
                nx2 = states[p + 2] if p + 2 < PAIRS else None
                if nx2 is not None:
                    # qp2 pool buffers rotate with bufs=2: pair p+2 reuses
                    # pair p's buffer whose pads are still zero -> no borders.
                    # qp1 including pads arrives fully by DMA.
                    dma_x(nx2)
                    dma_q1(nx2)
                last = (p == PAIRS - 1)
                rot = [[nc.gpsimd], [nc.sync], [nc.gpsimd]]
                for j in range(NCHUNK):
                    if last and j % 2 == 1:
                        se = ([nc.sync, nc.gpsimd] if j == NCHUNK - 1
                              else rot[(j // 2) % 3])
                    else:
                        se = None
                    conv2_chunk(cur, j, store_engines=se)
                    if j % 2 == 1:
                        g = j // 2
                        if nx1 is not None:
                            conv1_gen(nx1, g)

    nc.compile()
    return nc


def _get_nc():
    if "nc" not in _CACHE:
        _CACHE["nc"] = _build_nc()
    return _CACHE["nc"]


def _prep_host_inputs(inputs):
    import concourse.mybir as mybir

    fp8np = mybir.dt.np(mybir.dt.float8e4)

    x = np.asarray(inputs["x"], dtype=np.float32).reshape(B, C, HW)
    t = np.clip(np.rint(x * 256.0), -32768.0, 32767.0).astype(np.int16)
    t = np.ascontiguousarray(t)
    # q1 = clip(floor((t+512)/1024), 0, 15), host-padded to the 66x66 conv
    # input plane (exact integer math; fp8e4 holds 0..15 exactly)
    q1 = np.clip((t.astype(np.int32) + 512) >> 10, 0, 15)
    q1p = np.zeros((B, C, H + 2, PW), np.float32)
    q1p[:, :, 1:H + 1, 1:W + 1] = q1.reshape(B, C, H, W)
    q1p = np.ascontiguousarray(
        q1p.reshape(B, C, (H + 2) * PW).astype(fp8np))

    def wprep(w):
        wt = np.ascontiguousarray(w, dtype=np.float32).reshape(C, C, 3, 3)
        wt = wt.transpose(1, 0, 2, 3)                  # [in, out, ky, kx]
        taps = [kk for pair in TAP_PAIRS for kk in pair] + [TAP_SINGLE]
        out = np.zeros((128, 9, 128), np.float32)
        for t, (ky, kx) in enumerate(taps):
            out[0:64, t, 0:64] = wt[:, :, ky, kx]
            out[64:128, t, 64:128] = wt[:, :, ky, kx]
        return np.ascontiguousarray(out.reshape(128, 9 * 128).astype(fp8np))

    w1t = wprep(inputs["w1"])
    w2t = wprep(inputs["w2"])

    s1 = np.asarray(inputs["bn1_scale"], dtype=np.float64)
    b1 = np.asarray(inputs["bn1_bias"], dtype=np.float64)
    s2 = np.asarray(inputs["bn2_scale"], dtype=np.float64)
    b2 = np.asarray(inputs["bn2_bias"], dtype=np.float64)
    # all exact dyadic rationals -> float32 conversion is exact
    sB = (s1 * 2.0 ** -11).astype(np.float32)
    bB = ((2.0 * b1 + 1.0) * 2.0 ** -12).astype(np.float32)
    sC = s2.astype(np.float32)
    bC = b2.astype(np.float32)
    eps = np.full(64, 2.0 ** -11, dtype=np.float32)
    zer = np.zeros(64, dtype=np.float32)
    pp = np.stack([sB, bB, sC, bC, eps, zer], axis=1)      # [64, 6]
    pp = np.ascontiguousarray(np.concatenate([pp, pp], axis=0))  # [128, 6]

    return t, q1p, w1t, w2t, pp


def kernel(**inputs):
    from concourse.bass_utils import run_bass_kernel_spmd

    t, q1p, w1t, w2t, pp = _prep_host_inputs(inputs)
    nc = _get_nc()
    in_maps = []
    for i in range(N_CORES):
        sl = slice(i * IMG_PER_CORE, (i + 1) * IMG_PER_CORE)
        in_maps.append({"t": np.ascontiguousarray(t[sl]),
                        "q1p": np.ascontiguousarray(q1p[sl]),
                        "w1t": w1t, "w2t": w2t, "pp": pp})

    trace = bool(int(os.environ.get("KERNEL_TRACE", "0")))
    res = run_bass_kernel_spmd(nc, in_maps, core_ids=list(range(N_CORES)),
                               trace=trace)
    _CACHE["last_results"] = res
    out = np.concatenate([r["out"] for r in res.results], axis=0)
    return (out.reshape(B, C, H, W).astype(np.float32)) / 256.0
